# revision 26
# baseline (speedup 1.0000x reference)
"""BlockSparseLinear on 8 TRN2 NeuronCores — block-sparse PE-tiled kernel.

Computes out = x @ W_dense.T + bias where W_dense is [4096, 4096] assembled
from 8192 nonzero 32x32 blocks (50% density).

Strategy:
  - Pure 8-way token sharding: each core gets 512 tokens and all 8192
    blocks; the sparsity pattern is shared so all cores run one SPMD
    program (required by run_bass_kernel_spmd).
  - Only nonzero blocks are computed, using PE 32-wide sub-array tiling.
    Measured cost model: every matmul instruction pays the serial
    LDWEIGHTS path (~27ns for a 32-col stationary at the 1.2 GHz NX),
    so blocks are packed into TALL stationaries: k-blocks c live at
    partition band perm[c]%4, slot perm[c]//4 of the x tile; a row with
    >=2 blocks in one slot becomes a single [128k, 32o] QUAD matmul
    (absent bands zeroed, one LDW covers up to 4 blocks, marginal cost
    ~53ns warm); lone blocks stay [32k, 32o] SINGLEs (~29.5ns,
    LDW-bound).  The c->(band,slot) bijection is re-optimized per group
    of 32 rows (pairwise-swap hill climb, ~1s host time) and x is
    shipped once per group ([NGRP, 128, 32, 512] bf16).
  - Rows are processed in 32 windows of 4 rows (one per PE column strip
    j).  Window pairs emit all quads first (round-robin over the 4
    PHYSICAL strips), then singles (band-interleaved) — quads and
    singles use different PE tiling modes which cannot overlap, so they
    are phase-grouped to limit mode switches (~0.3-0.7us each).
  - PSUM discipline: tile -> bank 4*(window parity) + lowest-band; any
    two writers of the same (bank, 32-partition slice) share a sub-array
    so hardware FIFO serializes them; different slices of one bank may
    be written concurrently.  Dummy zero-weight singles cover (row,
    band) groups that would otherwise leave a psum slice unstarted.
  - DVE chains the 4 partial banks + bias per window; output is DMA'd
    as bf16 (precision budget allows it) and assembled on host.
  - DMA: weights stream in 512KB chunks on the SP ring (10-deep
    prefetch); per-group x is spread across both rings one chunk per
    window-pair, with pool bufs sized so prefetch DMAs never wait-block
    a FIFO queue head (bufs >= prefetch depth!).

Measured on the 8-core axon TRN2 pod: ~231us at the 2.0 GHz P0 clock
(~198us when the PE runs warm at 2.4 GHz); rel err 2.9e-3 vs the fp32
reference (bf16 inputs + bf16 output DMA).  Baseline dense kernel:
258-304us.  BSL_EMU=1 runs a numpy emulation of the exact schedule.
"""

import os
from itertools import permutations

import numpy as np
from ml_dtypes import bfloat16

import concourse.mybir as mybir
import concourse.tile as tile
from concourse import bacc
from concourse.bass_utils import run_bass_kernel_spmd

BLOCK = 32
IN_FEATURES = 4096
OUT_FEATURES = 4096
N_TOKENS = 4096
IB = IN_FEATURES // BLOCK  # 128 block-cols
OB = OUT_FEATURES // BLOCK  # 128 block-rows

N_CORES = 8
TSH = N_TOKENS // N_CORES  # 512 tokens per core
NFREE = 512
P = 128

NWIN = 32  # windows of 4 rows
NGRP = 4  # row groups; each group gets its own c->(band,slot) map + x copy
WPG = NWIN // NGRP  # windows per group
CHUNK_SLOTS = 64  # 32-col weight slots per DMA chunk
X_CHUNKS = 8  # x DMA chunks per group

QUAD_NS = 53.3  # measured marginal cost of a [128,32] quad matmul
SINGLE_NS = 29.5  # measured marginal cost of a [32,32] single matmul

LAST_EXEC_NS = None
LAST_RESULT = None


def _install_axon_ntff_hook():
    try:
        from antenv.axon_hooks import get_axon_ntff_profile_hook

        return get_axon_ntff_profile_hook() is not None
    except ImportError:
        pass
    try:
        import sys
        import types

        import antenv
        import trn_agent_boot.trn_boot as tb

        hook = tb._ntff_profile_via_ctypes("/opt/axon/libaxon_pjrt.so")
        if hook is None:
            return False
        mod = types.ModuleType("antenv.axon_hooks")
        mod._hook = hook
        mod.get_axon_ntff_profile_hook = lambda: mod._hook
        mod.set_axon_ntff_profile_hook = lambda h: setattr(mod, "_hook", h)
        sys.modules["antenv.axon_hooks"] = mod
        antenv.axon_hooks = mod

        import concourse.bass_utils as bu

        bu.upload_artifacts = lambda tmpdir: str(tmpdir)
        return True
    except Exception:
        return False


class _Tile:
    """One PE instruction: a QUAD ([128,32] stationary) or SINGLE ([32,32])."""

    __slots__ = ("kind", "row", "m", "blocks", "j", "w", "slot", "start", "stop")

    def __init__(self, kind, row, m, blocks):
        self.kind = kind  # 'q' or 's'
        self.row = row
        self.m = m
        self.blocks = blocks  # list of (band, bidx); bidx -1 = zero dummy
        self.j = -1
        self.w = -1
        self.slot = -1
        self.start = False
        self.stop = False

    @property
    def lowband(self):
        return 0 if self.kind == "q" else self.blocks[0][0]


def _opt_map(rowcs):
    """Optimize the c -> position bijection (slot = pos//4, band = pos%4) for
    a set of rows, minimizing quad/single cost: pairwise-swap hill climb with
    perturbation restarts (keep-best, deterministic)."""
    R = len(rowcs)
    M = np.zeros((R, IB), dtype=bool)
    for i, cs in enumerate(rowcs):
        M[i, cs] = True
    Mf = M.astype(np.float64)
    rows_idx = np.arange(R)[:, None]

    def colcost(v):
        return QUAD_NS * (v >= 2) + SINGLE_NS * (v == 1)

    def total_cost(perm):
        tot = 0.0
        for cs in rowcs:
            occ = np.bincount(perm[cs] // 4, minlength=32)
            tot += QUAD_NS * np.sum(occ >= 2) + SINGLE_NS * np.sum(occ == 1)
        return tot

    def climb(perm):
        occ = np.zeros((R, 32), dtype=np.int64)
        for i in range(R):
            occ[i] = np.bincount(perm[rowcs[i]] // 4, minlength=32)
        for sweep in range(8):
            improved = False
            for a in range(IB):
                sa = perm[a] // 4
                da = Mf[:, a]
                sb_all = perm // 4
                occ_sa = occ[:, sa][:, None] - da[:, None] + Mf
                occ_sb = occ[rows_idx, sb_all[None, :]] - Mf + da[:, None]
                delta = (
                    colcost(occ_sa)
                    + colcost(occ_sb)
                    - colcost(occ[:, sa])[:, None]
                    - colcost(occ[rows_idx, sb_all[None, :]])
                ).sum(axis=0)
                delta[sb_all == sa] = 0.0
                b = int(np.argmin(delta))
                if delta[b] < -1e-9:
                    sb = perm[b] // 4
                    occ[:, sa] += M[:, b].astype(np.int64) - M[:, a]
                    occ[:, sb] += M[:, a].astype(np.int64) - M[:, b]
                    perm[a], perm[b] = perm[b], perm[a]
                    improved = True
            if not improved:
                break
        return perm

    rng = np.random.default_rng(12345)
    best_perm = climb(np.arange(IB))
    best_cost = total_cost(best_perm)
    for _ in range(3):
        pert = best_perm.copy()
        idx = rng.choice(IB, 20, replace=False)
        vals = pert[idx].copy()
        rng.shuffle(vals)
        pert[idx] = vals
        cand = climb(pert)
        cc = total_cost(cand)
        if cc < best_cost - 1e-9:
            best_cost = cc
            best_perm = cand
    return best_perm


def _schedule(block_ids):
    ids = np.asarray(block_ids, dtype=np.int64)
    r_all = ids // IB
    c_all = ids % IB

    row_blocks = [[] for _ in range(OB)]
    for b in range(len(ids)):
        row_blocks[int(r_all[b])].append((int(c_all[b]), b))

    # preliminary window grouping by natural-map cost (units)
    nat_units = np.zeros(OB)
    for r in range(OB):
        occ = np.bincount([c // 4 for c, _ in row_blocks[r]], minlength=32)
        nat_units[r] = 4 * np.sum(occ >= 2) + np.sum(occ == 1)
    order = np.argsort(-nat_units, kind="stable")
    win_rows = [order[4 * w : 4 * w + 4] for w in range(NWIN)]

    # per-group map optimization, then per-row tiles under that map
    perms = []
    row_tiles = [None] * OB
    for g in range(NGRP):
        grows = [int(r) for w in range(WPG * g, WPG * (g + 1)) for r in win_rows[w]]
        rowcs = [np.array([c for c, _ in row_blocks[r]], dtype=np.int64) for r in grows]
        perm = _opt_map(rowcs)
        perms.append(perm)
        for r in grows:
            slots = {}
            for c, bidx in row_blocks[r]:
                pos = int(perm[c])
                slots.setdefault(pos // 4, []).append((pos % 4, bidx))
            tiles = []
            for m in sorted(slots):
                blks = sorted(slots[m])
                if len(blks) == 1:
                    band, bidx = blks[0]
                    tiles.append(_Tile("s", r, m, [(band, bidx)]))
                else:
                    tiles.append(_Tile("q", r, m, blks))
            havebands = set(t.lowband for t in tiles)
            for b in range(4):
                if b not in havebands:
                    tiles.append(_Tile("s", r, 0, [(b, -1)]))
            row_tiles[r] = tiles

    units = np.array(
        [sum(4 if t.kind == "q" else 1 for t in tiles) for tiles in row_tiles]
    )
    tot_u = np.zeros(4)
    tot_q = np.zeros(4)
    pair_q = np.zeros(4)  # quad count per strip within the current pair
    assign = np.zeros((NWIN, 4), dtype=np.int64)  # [w, j] -> row
    for w in range(NWIN):
        if w % 2 == 0:
            pair_q[:] = 0.0
        rows4 = win_rows[w]
        u4 = units[rows4].astype(float)
        q4 = np.array(
            [sum(1 for t in row_tiles[r] if t.kind == "q") for r in rows4],
            dtype=float,
        )
        best = None
        for perm in permutations(range(4)):
            tu = tot_u.copy()
            tq = tot_q.copy()
            pq = pair_q.copy()
            for k in range(4):
                tu[perm[k]] += u4[k]
                tq[perm[k]] += q4[k]
                pq[perm[k]] += q4[k]
            score = (pq.max() - pq.min(), tq.max() - tq.min(), tu.max() - tu.min())
            if best is None or score < best[0]:
                best = (score, perm)
        perm = best[1]
        for k in range(4):
            pair_q[perm[k]] += q4[k]
        for k in range(4):
            j = perm[k]
            r = int(rows4[k])
            assign[w, j] = r
            for t in row_tiles[r]:
                t.j = j
                t.w = w
            tot_u[j] += u4[k]
            tot_q[j] += q4[k]

    # emission order + slot assignment; windows are processed in PAIRS so
    # each quad->single tiling-mode switch covers two windows (8 psum banks)
    emit = []  # list over window-pairs of instruction lists
    slot_base = 0
    for wp in range(NWIN // 2):
        wl = []
        # 4 PHYSICAL strip queues (both windows of the pair concatenated):
        # round-robin must cycle hardware strips, not (window, strip) pairs,
        # or head-of-queue blocking kills quad concurrency.
        qlists = [[], [], [], []]
        slists = [[], [], [], []]
        for w in (2 * wp, 2 * wp + 1):
            for j in range(4):
                r = int(assign[w, j])
                strips = row_tiles[r]
                qlists[j].extend(t for t in strips if t.kind == "q")
                ss = [t for t in strips if t.kind == "s"]
                # band-interleave this strip's singles
                byband = [[], [], [], []]
                for t in ss:
                    byband[t.lowband].append(t)
                k = 0
                while any(byband):
                    b = k % 4
                    if byband[b]:
                        slists[j].append(byband[b].pop(0))
                    k += 1
        qi = [0] * 4
        while True:
            prog = False
            for j in range(4):
                if qi[j] < len(qlists[j]):
                    t = qlists[j][qi[j]]
                    t.slot = slot_base
                    slot_base += 1
                    wl.append(t)
                    qi[j] += 1
                    prog = True
            if not prog:
                break
        lane = [0, 0, 0, 0]  # per-band lane counter for singles slots
        si = [0] * 4
        sl = []
        while True:
            prog = False
            for j in range(4):
                if si[j] < len(slists[j]):
                    t = slists[j][si[j]]
                    b = t.lowband
                    t.slot = slot_base + lane[b]
                    lane[b] += 1
                    sl.append(t)
                    si[j] += 1
                    prog = True
            if not prog:
                break
        slot_base += max(lane) if any(lane) else 0
        wl.extend(sl)
        emit.append(wl)

    # start/stop flags per (row, lowband) group, in emission order
    groups = {}
    for wl in emit:
        for t in wl:
            groups.setdefault((t.row, t.lowband), []).append(t)
    for key, ts in groups.items():
        ts[0].start = True
        ts[-1].stop = True

    n_slots = slot_base
    nch = (n_slots + CHUNK_SLOTS - 1) // CHUNK_SLOTS
    return {
        "emit": emit,
        "assign": assign,
        "perms": perms,
        "NCH": nch,
        "n_slots": n_slots,
        "n_instr": sum(len(wl) for wl in emit),
    }


def _build_bass(sched):
    nch = sched["NCH"]
    emit = sched["emit"]

    nc = bacc.Bacc(None, target_bir_lowering=False)

    x_d = nc.dram_tensor(
        "xh", [NGRP, P, 32, TSH], mybir.dt.bfloat16, kind="ExternalInput"
    )
    w_d = nc.dram_tensor(
        "wst", [nch, P, CHUNK_SLOTS * BLOCK], mybir.dt.bfloat16, kind="ExternalInput"
    )
    b_d = nc.dram_tensor("biasq", [P, NWIN], mybir.dt.float32, kind="ExternalInput")
    o_d = nc.dram_tensor(
        "out", [NWIN, P, TSH], mybir.dt.bfloat16, kind="ExternalOutput"
    )

    with tile.TileContext(nc) as tc:
        with (
            tc.tile_pool(name="xpool", bufs=3) as xpool,
            tc.tile_pool(name="wpool", bufs=10) as wpool,
            tc.tile_pool(name="spool", bufs=4) as spool,
            tc.tile_pool(name="bpool", bufs=1) as bpool,
            tc.tile_pool(name="psum", bufs=2, space="PSUM") as ppool,
        ):
            bias_sb = bpool.tile([P, NWIN], mybir.dt.float32)
            nc.scalar.dma_start(bias_sb[:], b_d[:])

            # x per (group, chunk) tiles: each row-group has its own x copy
            # (custom c->slot map); chunk-granular deps keep the ramp short.
            MCH = 32 // X_CHUNKS
            x_tiles = {}

            def issue_x(g, xc, eng):
                if g < NGRP and (g, xc) not in x_tiles:
                    xt = xpool.tile(
                        [P, MCH, TSH], mybir.dt.bfloat16, tag=f"x{xc}", name="x"
                    )
                    eng.dma_start(
                        xt[:], x_d[g, :, xc * MCH : (xc + 1) * MCH, :]
                    )
                    x_tiles[(g, xc)] = xt

            w_tiles = {}
            W_PREFETCH = 8

            def issue_w(ch):
                if ch < nch and ch not in w_tiles:
                    w_sb = wpool.tile(
                        [P, CHUNK_SLOTS * BLOCK], mybir.dt.bfloat16, tag="w", name="w"
                    )
                    nc.sync.dma_start(w_sb[:], w_d[ch])
                    w_tiles[ch] = w_sb

            # group 0 x and early w interleaved across the two rings
            issue_x(0, 0, nc.scalar)
            issue_w(0)
            issue_x(0, 1, nc.sync)
            issue_x(0, 2, nc.scalar)
            issue_w(1)
            issue_x(0, 3, nc.sync)
            issue_x(0, 4, nc.scalar)
            issue_w(2)
            issue_x(0, 5, nc.sync)
            issue_x(0, 6, nc.scalar)
            issue_x(0, 7, nc.sync)
            for ch in range(3, W_PREFETCH + 1):
                issue_w(ch)

            ps_tiles = {}
            win_left = [16] * NWIN
            issued_ch = 0

            for wp, wl in enumerate(emit):
                g = (2 * wp) // WPG
                # prefetch next group's x spread across this group's pairs
                # (2 of 8 chunks per pair, alternating rings)
                pidx = wp % (WPG // 2)
                issue_x(g + 1, 2 * pidx, nc.scalar)
                issue_x(g + 1, 2 * pidx + 1, nc.sync)
                for t in wl:
                    ch = t.slot // CHUNK_SLOTS
                    while issued_ch < ch:
                        issued_ch += 1
                        issue_w(issued_ch + W_PREFETCH)
                    w_sb = w_tiles[ch]
                    col0 = (t.slot % CHUNK_SLOTS) * BLOCK
                    key = (t.w, t.lowband)
                    if key not in ps_tiles:
                        ps_tiles[key] = ppool.tile(
                            [P, NFREE],
                            mybir.dt.float32,
                            tag=f"ps{t.lowband}",
                            name="ps",
                        )
                    psum_t = ps_tiles[key]
                    j = t.j
                    x_sb = x_tiles[(t.w // WPG, t.m // MCH)]
                    mloc = t.m % MCH
                    if t.kind == "q":
                        nc.tensor.matmul(
                            psum_t[32 * j : 32 * j + 32, :],
                            lhsT=w_sb[:, col0 : col0 + BLOCK],
                            rhs=x_sb[:, mloc, :],
                            start=t.start,
                            stop=t.stop,
                            tile_position=(0, 32 * j),
                        )
                    else:
                        b = t.lowband
                        nc.tensor.matmul(
                            psum_t[32 * j : 32 * j + 32, :],
                            lhsT=w_sb[32 * b : 32 * b + 32, col0 : col0 + BLOCK],
                            rhs=x_sb[32 * b : 32 * b + 32, mloc, :],
                            start=t.start,
                            stop=t.stop,
                            tile_position=(32 * b, 32 * j),
                        )
                    if t.stop:
                        win_left[t.w] -= 1
                        if win_left[t.w] == 0:
                            q = t.w
                            pt = [ps_tiles.pop((q, ii)) for ii in range(4)]
                            s1 = spool.tile([P, NFREE], mybir.dt.float32, tag="s1")
                            nc.vector.tensor_tensor(
                                s1[:],
                                pt[0][:],
                                bias_sb[:, q : q + 1].to_broadcast([P, NFREE]),
                                mybir.AluOpType.add,
                            )
                            s2 = spool.tile([P, NFREE], mybir.dt.float32, tag="s2")
                            nc.vector.tensor_tensor(
                                s2[:], pt[1][:], s1[:], mybir.AluOpType.add
                            )
                            s3 = spool.tile([P, NFREE], mybir.dt.float32, tag="s3")
                            nc.vector.tensor_tensor(
                                s3[:], pt[2][:], s2[:], mybir.AluOpType.add
                            )
                            so = spool.tile([P, NFREE], mybir.dt.bfloat16, tag="so")
                            nc.vector.tensor_tensor(
                                so[:], pt[3][:], s3[:], mybir.AluOpType.add
                            )
                            nc.scalar.dma_start(o_d[q], so[:])

    nc.compile()
    return nc


def _prep_weights(weight_data, sched):
    nch = sched["NCH"]
    wdT = np.ascontiguousarray(weight_data.transpose(0, 2, 1)).astype(bfloat16)
    w_np = np.zeros((nch, P, CHUNK_SLOTS * BLOCK), dtype=bfloat16)
    for wl in sched["emit"]:
        for t in wl:
            ch = t.slot // CHUNK_SLOTS
            col0 = (t.slot % CHUNK_SLOTS) * BLOCK
            for band, bidx in t.blocks:
                if bidx >= 0:
                    w_np[ch, 32 * band : 32 * band + 32, col0 : col0 + BLOCK] = wdT[
                        bidx
                    ]
    return w_np


def _prep_x(x_shard, perms):
    """[TSH, 4096] f32 -> [NGRP, 128, 32, TSH] bf16; group g places k-block c
    at partition band perm_g[c]%4, slot perm_g[c]//4."""
    a = np.ascontiguousarray(x_shard.T).reshape(IB, BLOCK, TSH).astype(bfloat16)
    out = np.empty((NGRP, 4, BLOCK, 32, TSH), dtype=bfloat16)  # [g, band, q, m, t]
    for g, perm in enumerate(perms):
        inv = np.empty(IB, dtype=np.int64)
        inv[perm] = np.arange(IB)  # inv[pos] = c
        # position pos = 4m + band -> c = inv[pos]
        for band in range(4):
            cs = inv[np.arange(32) * 4 + band]  # c for each m at this band
            out[g, band] = a[cs].transpose(1, 0, 2)  # [q, m, t]
    return np.ascontiguousarray(out.reshape(NGRP, P, 32, TSH))


def _prep_bias(bias, sched):
    assign = sched["assign"]
    bias_np = np.zeros((P, NWIN), dtype=np.float32)
    for q in range(NWIN):
        for j in range(4):
            r = int(assign[q, j])
            bias_np[32 * j : 32 * j + 32, q] = bias[32 * r : 32 * r + 32]
    return bias_np


def _assemble_out(o_cores, sched):
    assign = sched["assign"]
    rflat = assign.reshape(-1)
    out = np.empty((N_TOKENS, OUT_FEATURES), dtype=np.float32)
    for core, o in enumerate(o_cores):
        o4 = np.asarray(o, dtype=np.float32).reshape(NWIN, 4, BLOCK, TSH)
        flat = o4.transpose(3, 0, 1, 2).reshape(TSH, OB, BLOCK)
        view = out[core * TSH : (core + 1) * TSH].reshape(TSH, OB, BLOCK)
        view[:, rflat, :] = flat
    return out


def _emulate_core(xh, w_np, bias_np, sched):
    o_d = np.zeros((NWIN, P, TSH), dtype=np.float32)
    psum = {}
    for wl in sched["emit"]:
        for t in wl:
            key = (t.w, t.lowband)
            if key not in psum:
                psum[key] = np.zeros((P, NFREE), dtype=np.float32)
            if t.start:
                psum[key][32 * t.j : 32 * t.j + 32, :] = 0.0
            ch = t.slot // CHUNK_SLOTS
            col0 = (t.slot % CHUNK_SLOTS) * BLOCK
            g = t.w // WPG
            if t.kind == "q":
                lhsT = w_np[ch, :, col0 : col0 + BLOCK].astype(np.float32)
                rhs = xh[g, :, t.m, :].astype(np.float32)
            else:
                b = t.lowband
                lhsT = w_np[ch, 32 * b : 32 * b + 32, col0 : col0 + BLOCK].astype(
                    np.float32
                )
                rhs = xh[g, 32 * b : 32 * b + 32, t.m, :].astype(np.float32)
            psum[key][32 * t.j : 32 * t.j + 32, :] += lhsT.T @ rhs
    for q in range(NWIN):
        acc = sum(psum[(q, i)] for i in range(4))
        o_d[q] = acc + bias_np[:, q : q + 1]
    return o_d


def kernel(x, weight_data, bias, block_ids):
    x = np.ascontiguousarray(np.asarray(x, dtype=np.float32))
    weight_data = np.asarray(weight_data, dtype=np.float32)
    bias = np.asarray(bias, dtype=np.float32)
    block_ids = np.asarray(block_ids)

    sched = _schedule(block_ids)
    w_np = _prep_weights(weight_data, sched)
    bias_np = _prep_bias(bias, sched)
    xhs = [_prep_x(x[c * TSH : (c + 1) * TSH], sched["perms"]) for c in range(N_CORES)]

    if bool(int(os.environ.get("BSL_EMU", "0"))):
        o_cores = [_emulate_core(xh, w_np, bias_np, sched) for xh in xhs]
        return _assemble_out(o_cores, sched)

    in_maps = [{"xh": xhs[c], "wst": w_np, "biasq": bias_np} for c in range(N_CORES)]

    nc = _build_bass(sched)
    trace = bool(int(os.environ.get("BSL_TRACE", "0")))
    if trace:
        trace = _install_axon_ntff_hook()
    kwargs = {}
    if trace:
        tdir = os.environ.get("BSL_TRACE_DIR")
        if tdir:
            os.makedirs(tdir, exist_ok=True)
            kwargs["tmpdir"] = tdir
        kwargs["trace_cores"] = list(range(N_CORES))
    res = run_bass_kernel_spmd(
        nc,
        in_maps,
        core_ids=list(range(N_CORES)),
        trace=trace,
        **kwargs,
    )

    global LAST_EXEC_NS, LAST_RESULT
    LAST_EXEC_NS = res.exec_time_ns
    LAST_RESULT = res

    o_cores = [res.results[c]["out"] for c in range(N_CORES)]
    return _assemble_out(o_cores, sched)


# revision 27
# speedup vs baseline: 1.1664x; 1.1664x over previous
"""BlockSparseLinear on 8 TRN2 NeuronCores — block-sparse PE-tiled kernel.

Computes out = x @ W_dense.T + bias where W_dense is [4096, 4096] assembled
from 8192 nonzero 32x32 blocks (50% density).

Strategy:
  - Pure 8-way token sharding: each core gets 512 tokens and all 8192
    blocks; the sparsity pattern is shared so all cores run one SPMD
    program (required by run_bass_kernel_spmd).
  - Only nonzero blocks are computed, using PE 32-wide sub-array tiling.
    Measured cost model: every matmul instruction pays the serial
    LDWEIGHTS path (~27ns for a 32-col stationary at the 1.2 GHz NX),
    so blocks are packed into TALL stationaries: k-blocks c live at
    partition band perm[c]%4, slot perm[c]//4 of the x tile; a row with
    >=2 blocks in one slot becomes a single [128k, 32o] QUAD matmul
    (absent bands zeroed, one LDW covers up to 4 blocks, marginal cost
    ~53ns warm); lone blocks stay [32k, 32o] SINGLEs (~29.5ns,
    LDW-bound).  The c->(band,slot) bijection is re-optimized per group
    of 32 rows (pairwise-swap hill climb, ~1s host time) and x is
    shipped once per group ([NGRP, 128, 32, 512] bf16).
  - Rows are processed in 32 windows of 4 rows (one per PE column strip
    j).  Window pairs emit all quads first (round-robin over the 4
    PHYSICAL strips), then singles (band-interleaved) — quads and
    singles use different PE tiling modes which cannot overlap, so they
    are phase-grouped to limit mode switches (~0.3-0.7us each).
  - PSUM discipline: tile -> bank 4*(window parity) + lowest-band; any
    two writers of the same (bank, 32-partition slice) share a sub-array
    so hardware FIFO serializes them; different slices of one bank may
    be written concurrently.  Dummy zero-weight singles cover (row,
    band) groups that would otherwise leave a psum slice unstarted.
  - DVE chains the 4 partial banks + bias per window; output is DMA'd
    as bf16 (precision budget allows it) and assembled on host.
  - DMA: weights stream in 512KB chunks on the SP ring (10-deep
    prefetch); per-group x is spread across both rings one chunk per
    window-pair, with pool bufs sized so prefetch DMAs never wait-block
    a FIFO queue head (bufs >= prefetch depth!).

Measured on the 8-core axon TRN2 pod: ~231us at the 2.0 GHz P0 clock
(~198us when the PE runs warm at 2.4 GHz); rel err 2.9e-3 vs the fp32
reference (bf16 inputs + bf16 output DMA).  Baseline dense kernel:
258-304us.  BSL_EMU=1 runs a numpy emulation of the exact schedule.
"""

import os
from itertools import permutations

import numpy as np
from ml_dtypes import bfloat16

import concourse.mybir as mybir
import concourse.tile as tile
from concourse import bacc
from concourse.bass_utils import run_bass_kernel_spmd

BLOCK = 32
IN_FEATURES = 4096
OUT_FEATURES = 4096
N_TOKENS = 4096
IB = IN_FEATURES // BLOCK  # 128 block-cols
OB = OUT_FEATURES // BLOCK  # 128 block-rows

N_CORES = 8
TSH = N_TOKENS // N_CORES  # 512 tokens per core
NFREE = 512
P = 128

NWIN = 32  # windows of 4 rows
NGRP = 4  # row groups; each group gets its own c->(band,slot) map + x copy
WPG = NWIN // NGRP  # windows per group
CHUNK_SLOTS = 64  # 32-col weight slots per DMA chunk
XB = [0, 2, 4, 8, 12, 16, 20, 24, 28, 32]  # x chunk m-boundaries (first two tiny)

QUAD_NS = 53.3  # measured marginal cost of a [128,32] quad matmul
SINGLE_NS = 29.5  # measured marginal cost of a [32,32] single matmul

LAST_EXEC_NS = None
LAST_RESULT = None


def _install_axon_ntff_hook():
    try:
        from antenv.axon_hooks import get_axon_ntff_profile_hook

        return get_axon_ntff_profile_hook() is not None
    except ImportError:
        pass
    try:
        import sys
        import types

        import antenv
        import trn_agent_boot.trn_boot as tb

        hook = tb._ntff_profile_via_ctypes("/opt/axon/libaxon_pjrt.so")
        if hook is None:
            return False
        mod = types.ModuleType("antenv.axon_hooks")
        mod._hook = hook
        mod.get_axon_ntff_profile_hook = lambda: mod._hook
        mod.set_axon_ntff_profile_hook = lambda h: setattr(mod, "_hook", h)
        sys.modules["antenv.axon_hooks"] = mod
        antenv.axon_hooks = mod

        import concourse.bass_utils as bu

        bu.upload_artifacts = lambda tmpdir: str(tmpdir)
        return True
    except Exception:
        return False


class _Tile:
    """One PE instruction: a QUAD ([128,32] stationary) or SINGLE ([32,32])."""

    __slots__ = ("kind", "row", "m", "blocks", "j", "w", "slot", "start", "stop")

    def __init__(self, kind, row, m, blocks):
        self.kind = kind  # 'q' or 's'
        self.row = row
        self.m = m
        self.blocks = blocks  # list of (band, bidx); bidx -1 = zero dummy
        self.j = -1
        self.w = -1
        self.slot = -1
        self.start = False
        self.stop = False

    @property
    def lowband(self):
        return 0 if self.kind == "q" else self.blocks[0][0]


def _opt_map(rowcs):
    """Optimize the c -> position bijection (slot = pos//4, band = pos%4) for
    a set of rows, minimizing quad/single cost: pairwise-swap hill climb with
    perturbation restarts (keep-best, deterministic)."""
    R = len(rowcs)
    M = np.zeros((R, IB), dtype=bool)
    for i, cs in enumerate(rowcs):
        M[i, cs] = True
    Mf = M.astype(np.float64)
    rows_idx = np.arange(R)[:, None]

    def colcost(v):
        return QUAD_NS * (v >= 2) + SINGLE_NS * (v == 1)

    def total_cost(perm):
        tot = 0.0
        for cs in rowcs:
            occ = np.bincount(perm[cs] // 4, minlength=32)
            tot += QUAD_NS * np.sum(occ >= 2) + SINGLE_NS * np.sum(occ == 1)
        return tot

    def climb(perm):
        occ = np.zeros((R, 32), dtype=np.int64)
        for i in range(R):
            occ[i] = np.bincount(perm[rowcs[i]] // 4, minlength=32)
        for sweep in range(8):
            improved = False
            for a in range(IB):
                sa = perm[a] // 4
                da = Mf[:, a]
                sb_all = perm // 4
                occ_sa = occ[:, sa][:, None] - da[:, None] + Mf
                occ_sb = occ[rows_idx, sb_all[None, :]] - Mf + da[:, None]
                delta = (
                    colcost(occ_sa)
                    + colcost(occ_sb)
                    - colcost(occ[:, sa])[:, None]
                    - colcost(occ[rows_idx, sb_all[None, :]])
                ).sum(axis=0)
                delta[sb_all == sa] = 0.0
                b = int(np.argmin(delta))
                if delta[b] < -1e-9:
                    sb = perm[b] // 4
                    occ[:, sa] += M[:, b].astype(np.int64) - M[:, a]
                    occ[:, sb] += M[:, a].astype(np.int64) - M[:, b]
                    perm[a], perm[b] = perm[b], perm[a]
                    improved = True
            if not improved:
                break
        return perm

    rng = np.random.default_rng(12345)
    best_perm = climb(np.arange(IB))
    best_cost = total_cost(best_perm)
    for _ in range(3):
        pert = best_perm.copy()
        idx = rng.choice(IB, 20, replace=False)
        vals = pert[idx].copy()
        rng.shuffle(vals)
        pert[idx] = vals
        cand = climb(pert)
        cc = total_cost(cand)
        if cc < best_cost - 1e-9:
            best_cost = cc
            best_perm = cand
    return best_perm


def _schedule(block_ids):
    ids = np.asarray(block_ids, dtype=np.int64)
    r_all = ids // IB
    c_all = ids % IB

    row_blocks = [[] for _ in range(OB)]
    for b in range(len(ids)):
        row_blocks[int(r_all[b])].append((int(c_all[b]), b))

    # preliminary window grouping by natural-map cost (units)
    nat_units = np.zeros(OB)
    for r in range(OB):
        occ = np.bincount([c // 4 for c, _ in row_blocks[r]], minlength=32)
        nat_units[r] = 4 * np.sum(occ >= 2) + np.sum(occ == 1)
    order = np.argsort(-nat_units, kind="stable")
    win_rows = [order[4 * w : 4 * w + 4] for w in range(NWIN)]

    # per-group map optimization, then per-row tiles under that map
    perms = []
    row_tiles = [None] * OB
    for g in range(NGRP):
        grows = [int(r) for w in range(WPG * g, WPG * (g + 1)) for r in win_rows[w]]
        rowcs = [np.array([c for c, _ in row_blocks[r]], dtype=np.int64) for r in grows]
        perm = _opt_map(rowcs)
        perms.append(perm)
        for r in grows:
            slots = {}
            for c, bidx in row_blocks[r]:
                pos = int(perm[c])
                slots.setdefault(pos // 4, []).append((pos % 4, bidx))
            tiles = []
            for m in sorted(slots):
                blks = sorted(slots[m])
                if len(blks) == 1:
                    band, bidx = blks[0]
                    tiles.append(_Tile("s", r, m, [(band, bidx)]))
                else:
                    tiles.append(_Tile("q", r, m, blks))
            havebands = set(t.lowband for t in tiles)
            for b in range(4):
                if b not in havebands:
                    tiles.append(_Tile("s", r, 0, [(b, -1)]))
            row_tiles[r] = tiles

    units = np.array(
        [sum(4 if t.kind == "q" else 1 for t in tiles) for tiles in row_tiles]
    )
    tot_u = np.zeros(4)
    tot_q = np.zeros(4)
    pair_q = np.zeros(4)  # quad count per strip within the current pair
    assign = np.zeros((NWIN, 4), dtype=np.int64)  # [w, j] -> row
    for w in range(NWIN):
        if w % 2 == 0:
            pair_q[:] = 0.0
        rows4 = win_rows[w]
        u4 = units[rows4].astype(float)
        q4 = np.array(
            [sum(1 for t in row_tiles[r] if t.kind == "q") for r in rows4],
            dtype=float,
        )
        best = None
        for perm in permutations(range(4)):
            tu = tot_u.copy()
            tq = tot_q.copy()
            pq = pair_q.copy()
            for k in range(4):
                tu[perm[k]] += u4[k]
                tq[perm[k]] += q4[k]
                pq[perm[k]] += q4[k]
            score = (pq.max() - pq.min(), tq.max() - tq.min(), tu.max() - tu.min())
            if best is None or score < best[0]:
                best = (score, perm)
        perm = best[1]
        for k in range(4):
            pair_q[perm[k]] += q4[k]
        for k in range(4):
            j = perm[k]
            r = int(rows4[k])
            assign[w, j] = r
            for t in row_tiles[r]:
                t.j = j
                t.w = w
            tot_u[j] += u4[k]
            tot_q[j] += q4[k]

    # emission order + slot assignment; windows are processed in PAIRS so
    # each quad->single tiling-mode switch covers two windows (8 psum banks)
    emit = []  # list over window-pairs of instruction lists
    slot_base = 0
    for wp in range(NWIN // 2):
        wl = []
        # 4 PHYSICAL strip queues (both windows of the pair concatenated):
        # round-robin must cycle hardware strips, not (window, strip) pairs,
        # or head-of-queue blocking kills quad concurrency.
        qlists = [[], [], [], []]
        slists = [[], [], [], []]
        for w in (2 * wp, 2 * wp + 1):
            for j in range(4):
                r = int(assign[w, j])
                strips = row_tiles[r]
                qlists[j].extend(t for t in strips if t.kind == "q")
                ss = [t for t in strips if t.kind == "s"]
                # band-interleave this strip's singles
                byband = [[], [], [], []]
                for t in ss:
                    byband[t.lowband].append(t)
                k = 0
                while any(byband):
                    b = k % 4
                    if byband[b]:
                        slists[j].append(byband[b].pop(0))
                    k += 1
        qi = [0] * 4
        while True:
            prog = False
            for j in range(4):
                if qi[j] < len(qlists[j]):
                    t = qlists[j][qi[j]]
                    t.slot = slot_base
                    slot_base += 1
                    wl.append(t)
                    qi[j] += 1
                    prog = True
            if not prog:
                break
        lane = [0, 0, 0, 0]  # per-band lane counter for singles slots
        si = [0] * 4
        sl = []
        while True:
            prog = False
            for j in range(4):
                if si[j] < len(slists[j]):
                    t = slists[j][si[j]]
                    b = t.lowband
                    t.slot = slot_base + lane[b]
                    lane[b] += 1
                    sl.append(t)
                    si[j] += 1
                    prog = True
            if not prog:
                break
        slot_base += max(lane) if any(lane) else 0
        wl.extend(sl)
        emit.append(wl)

    # start/stop flags per (row, lowband) group, in emission order
    groups = {}
    for wl in emit:
        for t in wl:
            groups.setdefault((t.row, t.lowband), []).append(t)
    for key, ts in groups.items():
        ts[0].start = True
        ts[-1].stop = True

    n_slots = slot_base
    nch = (n_slots + CHUNK_SLOTS - 1) // CHUNK_SLOTS
    return {
        "emit": emit,
        "assign": assign,
        "perms": perms,
        "NCH": nch,
        "n_slots": n_slots,
        "n_instr": sum(len(wl) for wl in emit),
    }


def _build_bass(sched):
    nch = sched["NCH"]
    emit = sched["emit"]

    nc = bacc.Bacc(None, target_bir_lowering=False)

    x_d = nc.dram_tensor(
        "xh", [NGRP, P, 32, TSH], mybir.dt.bfloat16, kind="ExternalInput"
    )
    w_d = nc.dram_tensor(
        "wst", [nch, P, CHUNK_SLOTS * BLOCK], mybir.dt.bfloat16, kind="ExternalInput"
    )
    b_d = nc.dram_tensor("biasq", [P, NWIN], mybir.dt.float32, kind="ExternalInput")
    o_d = nc.dram_tensor(
        "out", [NWIN, P, TSH], mybir.dt.bfloat16, kind="ExternalOutput"
    )

    with tile.TileContext(nc) as tc:
        with (
            tc.tile_pool(name="xpool", bufs=3) as xpool,
            tc.tile_pool(name="wpool", bufs=10) as wpool,
            tc.tile_pool(name="spool", bufs=4) as spool,
            tc.tile_pool(name="bpool", bufs=1) as bpool,
            tc.tile_pool(name="psum", bufs=2, space="PSUM") as ppool,
        ):
            bias_sb = bpool.tile([P, NWIN], mybir.dt.float32)
            nc.scalar.dma_start(bias_sb[:], b_d[:])

            # x per (group, chunk) tiles: each row-group has its own x copy
            # (custom c->slot map).  Tile deps are whole-tile, so the first
            # chunks are tiny (2 slots) to let compute start early.
            x_tiles = {}
            m2chunk = np.searchsorted(np.array(XB), np.arange(32), side="right") - 1

            def issue_x(g, xc, eng):
                if g < NGRP and xc < len(XB) - 1 and (g, xc) not in x_tiles:
                    lo, hi = XB[xc], XB[xc + 1]
                    xt = xpool.tile(
                        [P, hi - lo, TSH], mybir.dt.bfloat16, tag=f"x{xc}", name="x"
                    )
                    eng.dma_start(xt[:], x_d[g, :, lo:hi, :])
                    x_tiles[(g, xc)] = xt

            w_tiles = {}
            W_PREFETCH = 8

            def issue_w(ch):
                if ch < nch and ch not in w_tiles:
                    w_sb = wpool.tile(
                        [P, CHUNK_SLOTS * BLOCK], mybir.dt.bfloat16, tag="w", name="w"
                    )
                    nc.sync.dma_start(w_sb[:], w_d[ch])
                    w_tiles[ch] = w_sb

            # group 0 x and early w interleaved across the two rings
            issue_x(0, 0, nc.scalar)
            issue_w(0)
            issue_x(0, 1, nc.sync)
            issue_x(0, 2, nc.scalar)
            issue_w(1)
            issue_x(0, 3, nc.sync)
            issue_x(0, 4, nc.scalar)
            issue_w(2)
            issue_x(0, 5, nc.sync)
            issue_x(0, 6, nc.scalar)
            issue_x(0, 7, nc.sync)
            issue_x(0, 8, nc.scalar)
            for ch in range(3, W_PREFETCH + 1):
                issue_w(ch)

            ps_tiles = {}
            win_left = [16] * NWIN
            issued_ch = 0

            for wp, wl in enumerate(emit):
                g = (2 * wp) // WPG
                # prefetch next group's x spread across this group's pairs
                pidx = wp % (WPG // 2)
                for k in range(3 * pidx, min(3 * pidx + 3, len(XB) - 1)):
                    issue_x(g + 1, k, nc.scalar if k % 2 == 0 else nc.sync)
                for t in wl:
                    ch = t.slot // CHUNK_SLOTS
                    while issued_ch < ch:
                        issued_ch += 1
                        issue_w(issued_ch + W_PREFETCH)
                    w_sb = w_tiles[ch]
                    col0 = (t.slot % CHUNK_SLOTS) * BLOCK
                    key = (t.w, t.lowband)
                    if key not in ps_tiles:
                        ps_tiles[key] = ppool.tile(
                            [P, NFREE],
                            mybir.dt.float32,
                            tag=f"ps{t.lowband}",
                            name="ps",
                        )
                    psum_t = ps_tiles[key]
                    j = t.j
                    xc = int(m2chunk[t.m])
                    x_sb = x_tiles[(t.w // WPG, xc)]
                    mloc = t.m - XB[xc]
                    if t.kind == "q":
                        nc.tensor.matmul(
                            psum_t[32 * j : 32 * j + 32, :],
                            lhsT=w_sb[:, col0 : col0 + BLOCK],
                            rhs=x_sb[:, mloc, :],
                            start=t.start,
                            stop=t.stop,
                            tile_position=(0, 32 * j),
                        )
                    else:
                        b = t.lowband
                        nc.tensor.matmul(
                            psum_t[32 * j : 32 * j + 32, :],
                            lhsT=w_sb[32 * b : 32 * b + 32, col0 : col0 + BLOCK],
                            rhs=x_sb[32 * b : 32 * b + 32, mloc, :],
                            start=t.start,
                            stop=t.stop,
                            tile_position=(32 * b, 32 * j),
                        )
                    if t.stop:
                        win_left[t.w] -= 1
                        if win_left[t.w] == 0:
                            q = t.w
                            pt = [ps_tiles.pop((q, ii)) for ii in range(4)]
                            s1 = spool.tile([P, NFREE], mybir.dt.float32, tag="s1")
                            nc.vector.tensor_tensor(
                                s1[:],
                                pt[0][:],
                                bias_sb[:, q : q + 1].to_broadcast([P, NFREE]),
                                mybir.AluOpType.add,
                            )
                            s2 = spool.tile([P, NFREE], mybir.dt.float32, tag="s2")
                            nc.vector.tensor_tensor(
                                s2[:], pt[1][:], s1[:], mybir.AluOpType.add
                            )
                            s3 = spool.tile([P, NFREE], mybir.dt.float32, tag="s3")
                            nc.vector.tensor_tensor(
                                s3[:], pt[2][:], s2[:], mybir.AluOpType.add
                            )
                            so = spool.tile([P, NFREE], mybir.dt.bfloat16, tag="so")
                            nc.vector.tensor_tensor(
                                so[:], pt[3][:], s3[:], mybir.AluOpType.add
                            )
                            nc.scalar.dma_start(o_d[q], so[:])

    nc.compile()
    return nc


def _prep_weights(weight_data, sched):
    nch = sched["NCH"]
    wdT = np.ascontiguousarray(weight_data.transpose(0, 2, 1)).astype(bfloat16)
    w_np = np.zeros((nch, P, CHUNK_SLOTS * BLOCK), dtype=bfloat16)
    for wl in sched["emit"]:
        for t in wl:
            ch = t.slot // CHUNK_SLOTS
            col0 = (t.slot % CHUNK_SLOTS) * BLOCK
            for band, bidx in t.blocks:
                if bidx >= 0:
                    w_np[ch, 32 * band : 32 * band + 32, col0 : col0 + BLOCK] = wdT[
                        bidx
                    ]
    return w_np


def _prep_x(x_shard, perms):
    """[TSH, 4096] f32 -> [NGRP, 128, 32, TSH] bf16; group g places k-block c
    at partition band perm_g[c]%4, slot perm_g[c]//4."""
    a = np.ascontiguousarray(x_shard.T).reshape(IB, BLOCK, TSH).astype(bfloat16)
    out = np.empty((NGRP, 4, BLOCK, 32, TSH), dtype=bfloat16)  # [g, band, q, m, t]
    for g, perm in enumerate(perms):
        inv = np.empty(IB, dtype=np.int64)
        inv[perm] = np.arange(IB)  # inv[pos] = c
        # position pos = 4m + band -> c = inv[pos]
        for band in range(4):
            cs = inv[np.arange(32) * 4 + band]  # c for each m at this band
            out[g, band] = a[cs].transpose(1, 0, 2)  # [q, m, t]
    return np.ascontiguousarray(out.reshape(NGRP, P, 32, TSH))


def _prep_bias(bias, sched):
    assign = sched["assign"]
    bias_np = np.zeros((P, NWIN), dtype=np.float32)
    for q in range(NWIN):
        for j in range(4):
            r = int(assign[q, j])
            bias_np[32 * j : 32 * j + 32, q] = bias[32 * r : 32 * r + 32]
    return bias_np


def _assemble_out(o_cores, sched):
    assign = sched["assign"]
    rflat = assign.reshape(-1)
    out = np.empty((N_TOKENS, OUT_FEATURES), dtype=np.float32)
    for core, o in enumerate(o_cores):
        o4 = np.asarray(o, dtype=np.float32).reshape(NWIN, 4, BLOCK, TSH)
        flat = o4.transpose(3, 0, 1, 2).reshape(TSH, OB, BLOCK)
        view = out[core * TSH : (core + 1) * TSH].reshape(TSH, OB, BLOCK)
        view[:, rflat, :] = flat
    return out


def _emulate_core(xh, w_np, bias_np, sched):
    o_d = np.zeros((NWIN, P, TSH), dtype=np.float32)
    psum = {}
    for wl in sched["emit"]:
        for t in wl:
            key = (t.w, t.lowband)
            if key not in psum:
                psum[key] = np.zeros((P, NFREE), dtype=np.float32)
            if t.start:
                psum[key][32 * t.j : 32 * t.j + 32, :] = 0.0
            ch = t.slot // CHUNK_SLOTS
            col0 = (t.slot % CHUNK_SLOTS) * BLOCK
            g = t.w // WPG
            if t.kind == "q":
                lhsT = w_np[ch, :, col0 : col0 + BLOCK].astype(np.float32)
                rhs = xh[g, :, t.m, :].astype(np.float32)
            else:
                b = t.lowband
                lhsT = w_np[ch, 32 * b : 32 * b + 32, col0 : col0 + BLOCK].astype(
                    np.float32
                )
                rhs = xh[g, 32 * b : 32 * b + 32, t.m, :].astype(np.float32)
            psum[key][32 * t.j : 32 * t.j + 32, :] += lhsT.T @ rhs
    for q in range(NWIN):
        acc = sum(psum[(q, i)] for i in range(4))
        o_d[q] = acc + bias_np[:, q : q + 1]
    return o_d


def kernel(x, weight_data, bias, block_ids):
    x = np.ascontiguousarray(np.asarray(x, dtype=np.float32))
    weight_data = np.asarray(weight_data, dtype=np.float32)
    bias = np.asarray(bias, dtype=np.float32)
    block_ids = np.asarray(block_ids)

    sched = _schedule(block_ids)
    w_np = _prep_weights(weight_data, sched)
    bias_np = _prep_bias(bias, sched)
    xhs = [_prep_x(x[c * TSH : (c + 1) * TSH], sched["perms"]) for c in range(N_CORES)]

    if bool(int(os.environ.get("BSL_EMU", "0"))):
        o_cores = [_emulate_core(xh, w_np, bias_np, sched) for xh in xhs]
        return _assemble_out(o_cores, sched)

    in_maps = [{"xh": xhs[c], "wst": w_np, "biasq": bias_np} for c in range(N_CORES)]

    nc = _build_bass(sched)
    trace = bool(int(os.environ.get("BSL_TRACE", "0")))
    if trace:
        trace = _install_axon_ntff_hook()
    kwargs = {}
    if trace:
        tdir = os.environ.get("BSL_TRACE_DIR")
        if tdir:
            os.makedirs(tdir, exist_ok=True)
            kwargs["tmpdir"] = tdir
        kwargs["trace_cores"] = list(range(N_CORES))
    res = run_bass_kernel_spmd(
        nc,
        in_maps,
        core_ids=list(range(N_CORES)),
        trace=trace,
        **kwargs,
    )

    global LAST_EXEC_NS, LAST_RESULT
    LAST_EXEC_NS = res.exec_time_ns
    LAST_RESULT = res

    o_cores = [res.results[c]["out"] for c in range(N_CORES)]
    return _assemble_out(o_cores, sched)
